# revision 3
# baseline (speedup 1.0000x reference)
"""Trainium2 Bass kernel for CustomRBF forward:

    out[i] = w * exp(-gamma * ||X[i] - centroid||^2) + b

Flat-contiguous layout (per core, data-parallel over 8 cores):
  - The 125056-sample shard is viewed flat as [128 partitions, 977 samples
    * 128 feats]: partition p holds samples [p*977, (p+1)*977) of the
    shard, each partition line a fully CONTIGUOUS 500,224-byte DRAM run.
    DMA groups of `ch` sample-columns load [128, ch*128] tiles whose
    per-partition reads are ch*512 contiguous bytes (32 KB at ch=64) --
    vs the 512-byte strided chunks of the natural [sample, feat] layout.
  - PE path per 128-sample column slice: TensorE transpose [p, 128] ->
    PSUM [feat, p]; ScalarE fused subtract+square (activation Square,
    bias=-c, per-partition = per-feature); TensorE fp32r matmul (squared
    slice stationary, ones moving) reduces over feature partitions ->
    one PSUM accumulator column [128, 1] that lands DIRECTLY in the
    [partition, sample-col] output layout (no transpose-back needed).
  - Optional DVE path (ratio knob) keeps whole slices on VectorE
    (tensor_sub/mul/reduce in natural flat layout) to rebalance engines.
  - Finalize once per repeat per path: ScalarE Exp (scale=-gamma) PSUM ->
    SBUF, VectorE tensor_scalar (*w + b), one output DMA per run of
    consecutive columns ([128, run] -> out[p*977 + c]).

Sharding: cores 0-6 take contiguous 125056-sample slices; core 7 takes the
last 125056 samples (overlapping core 6 by 448 samples; the overlap is
recomputed identically and overwritten at gather time).

`repeats` re-emits the whole pipeline R times in one NEFF (same data, same
output) -- used only for differential wall-clock timing of the steady state.
"""

import sys

sys.path.insert(0, "/opt/trn_rl_repo")

import numpy as np

D = 128          # feature dim
P = 128          # SBUF partitions
GAMMA = 1.0 / D
N_CORES = 8
SPP = 977        # samples per partition
SHARD = P * SPP  # 125056
N_TOTAL = 1000000
CH = 64          # sample-columns per input DMA
HG = 8           # sample-columns per half-group (one ACT square)

_NC_CACHE = {}


def _build(spp=SPP, repeats=1, ch=CH, hg=HG, xin_bufs=3, y_bufs=3,
           tr_bufs=2, acc_bufs=1, nmov=2, pe_num=1, den=1, depth=2):
    from contextlib import ExitStack

    import concourse.tile as tile
    from concourse import bacc, mybir

    f32 = mybir.dt.float32
    f32r = mybir.dt.float32r
    Act = mybir.ActivationFunctionType
    Alu = mybir.AluOpType

    nc = bacc.Bacc("TRN2", target_bir_lowering=False, debug=False,
                   num_devices=N_CORES)
    xh = nc.declare_dram_parameter("x", [P, spp * D], f32, isOutput=False)
    negch = nc.declare_dram_parameter("negc", [P, 1], f32, isOutput=False)
    identh = nc.declare_dram_parameter("ident", [P, D], f32, isOutput=False)
    onesh = nc.declare_dram_parameter("ones", [P, 2], f32, isOutput=False)
    creph = nc.declare_dram_parameter("crep", [P, HG * D], f32,
                                      isOutput=False)
    wh = nc.declare_dram_parameter("wvec", [P, 1], f32, isOutput=False)
    bh = nc.declare_dram_parameter("bvec", [P, 1], f32, isOutput=False)
    outh = nc.declare_dram_parameter("out", [P, spp], f32, isOutput=True)

    with ExitStack() as ctx:
        tc = ctx.enter_context(tile.TileContext(nc))
        singles = ctx.enter_context(tc.tile_pool(name="singles", bufs=1))
        xin = ctx.enter_context(tc.tile_pool(name="xin", bufs=xin_bufs))
        yp = ctx.enter_context(tc.tile_pool(name="y", bufs=y_bufs))
        dfp = ctx.enter_context(tc.tile_pool(name="df", bufs=2))
        vap = ctx.enter_context(tc.tile_pool(name="vacc", bufs=2))
        resp = ctx.enter_context(tc.tile_pool(name="res", bufs=2))
        trp = ctx.enter_context(tc.tile_pool(name="tr", bufs=tr_bufs,
                                             space="PSUM"))
        accp = ctx.enter_context(tc.tile_pool(name="acc", bufs=acc_bufs,
                                              space="PSUM"))

        negc_s = singles.tile([P, 1], f32)
        nc.sync.dma_start(out=negc_s, in_=negch[:, :])
        ident_s = singles.tile([P, D], f32)
        nc.sync.dma_start(out=ident_s, in_=identh[:, :])
        ones_s = singles.tile([P, 2], f32)
        nc.sync.dma_start(out=ones_s, in_=onesh[:, :])
        ones_r = singles.tile([P, 2], f32r)
        nc.vector.tensor_copy(out=ones_r, in_=ones_s)
        crep_s = singles.tile([P, HG * D], f32)
        nc.sync.dma_start(out=crep_s, in_=creph[:, :])
        crep3 = crep_s.rearrange("p (t k) -> p t k", k=D)
        wv_s = singles.tile([P, 1], f32)
        nc.sync.dma_start(out=wv_s, in_=wh[:, :])
        bv_s = singles.tile([P, 1], f32)
        nc.sync.dma_start(out=bv_s, in_=bh[:, :])

        # per-repeat state
        state = {}

        def start_repeat():
            state["acc"] = accp.tile([P, spp * nmov], f32, name="acc",
                                     tag="acc")
            state["vacc"] = vap.tile([P, spp], f32, name="vacc", tag="vacc")
            state["pe_runs"] = []   # merged (c0, len) runs owned by PE path
            state["v_runs"] = []
            state["pending"] = []   # deferred matmul closures

        def add_run(runs, c0, n):
            if runs and runs[-1][0] + runs[-1][1] == c0:
                runs[-1] = (runs[-1][0], runs[-1][1] + n)
            else:
                runs.append((c0, n))

        def finalize():
            acc, vacc = state["acc"], state["vacc"]
            if nmov == 2:
                acc3 = acc.rearrange("p (t two) -> p t two", two=2)
            for runs, kind in ((state["pe_runs"], "pe"),
                               (state["v_runs"], "v")):
                if not runs:
                    continue
                res = resp.tile([P, spp], f32, name="res", tag="res")
                for c0, n in runs:
                    if kind == "pe":
                        src = (acc3[:, c0:c0 + n, 0:1] if nmov == 2
                               else acc[:, c0:c0 + n])
                    else:
                        src = vacc[:, c0:c0 + n]
                    nc.scalar.activation(out=res[:, c0:c0 + n], in_=src,
                                         func=Act.Exp, scale=-GAMMA, bias=0.0)
                    nc.vector.tensor_scalar(out=res[:, c0:c0 + n],
                                            in0=res[:, c0:c0 + n],
                                            scalar1=wv_s[:, :],
                                            scalar2=bv_s[:, :],
                                            op0=Alu.mult, op1=Alu.add)
                    nc.sync.dma_start(out=outh[:, c0:c0 + n],
                                      in_=res[:, c0:c0 + n])

        def flush(all_=False):
            q = state["pending"]
            while q and (all_ or len(q) > depth):
                q.pop(0)()

        def pe_half(xt3, h, hw, col):
            tr = trp.tile([P, hg * D], f32, name="tr", tag="tr")
            for j in range(hw):
                nc.tensor.transpose(out=tr[:, j * D:(j + 1) * D],
                                    in_=xt3[:, h + j, :],
                                    identity=ident_s[:, :])
            y = yp.tile([P, hg * D], f32r, name="y", tag="y")
            nc.scalar.activation(out=y[:, :hw * D], in_=tr[:, :hw * D],
                                 func=Act.Square, bias=negc_s[:, :],
                                 scale=1.0)
            acc = state["acc"]

            def back():
                for j in range(hw):
                    c = (col + j) * nmov
                    nc.tensor.matmul(out=acc[:, c:c + nmov],
                                     lhsT=y[:, j * D:(j + 1) * D],
                                     rhs=ones_r[:, :nmov],
                                     start=True, stop=True)

            state["pending"].append(back)
            add_run(state["pe_runs"], col, hw)

        def v_half(xt3, h, hw, col):
            df = dfp.tile([P, hg, D], f32, name="df", tag="df")
            nc.vector.tensor_sub(out=df[:, :hw, :], in0=xt3[:, h:h + hw, :],
                                 in1=crep3[:, :hw, :])
            nc.vector.tensor_mul(out=df[:, :hw, :], in0=df[:, :hw, :],
                                 in1=df[:, :hw, :])
            nc.vector.tensor_reduce(out=state["vacc"][:, col:col + hw],
                                    in_=df[:, :hw, :],
                                    axis=mybir.AxisListType.X, op=Alu.add)
            add_run(state["v_runs"], col, hw)

        for _rep in range(repeats):
            start_repeat()
            hg_idx = 0
            col = 0
            while col < spp:
                cw = min(ch, spp - col)
                xt = xin.tile([P, ch * D], f32, name="xt", tag="xt")
                nc.sync.dma_start(out=xt[:, :cw * D],
                                  in_=xh[:, col * D:(col + cw) * D])
                xt3 = xt.rearrange("p (s k) -> p s k", k=D)
                h = 0
                while h < cw:
                    hw = min(hg, cw - h)
                    is_pe = (((hg_idx + 1) * pe_num) // den
                             > (hg_idx * pe_num) // den)
                    if is_pe:
                        flush()
                        pe_half(xt3, h, hw, col + h)
                    else:
                        v_half(xt3, h, hw, col + h)
                    hg_idx += 1
                    h += hw
                col += cw
            flush(all_=True)
            finalize()

    nc.finalize()
    return nc


def _get_nc():
    if "nc" not in _NC_CACHE:
        _NC_CACHE["nc"] = _build()
    return _NC_CACHE["nc"]


def _make_const_inputs(centroid, w, b):
    centroid = np.asarray(centroid, dtype=np.float32).reshape(D)
    w = np.asarray(w, dtype=np.float32).reshape(-1)[0]
    b = np.asarray(b, dtype=np.float32).reshape(-1)[0]
    return {
        "negc": (-centroid).reshape(P, 1).copy(),
        "ident": np.eye(P, dtype=np.float32),
        "ones": np.tile(np.array([1.0, 0.0], dtype=np.float32), (P, 1)),
        "crep": np.tile(np.tile(centroid, HG), (P, 1)),
        "wvec": np.full((P, 1), w, dtype=np.float32),
        "bvec": np.full((P, 1), b, dtype=np.float32),
    }


def _shard_x(x_shard):
    # [SHARD, D] sample-major -> flat [P, SPP*D]: partition p holds
    # samples [p*SPP, (p+1)*SPP) as one contiguous run (pure view).
    return np.ascontiguousarray(x_shard).reshape(P, SPP * D)


def kernel(X, centroid, w, b, _trace=False, _trace_kwargs=None):
    from concourse.bass_utils import run_bass_kernel_spmd

    X = np.asarray(X)
    assert X.shape == (N_TOTAL, D), X.shape
    if X.dtype != np.float32:
        X = X.astype(np.float32)

    consts = _make_const_inputs(centroid, w, b)
    starts = [i * SHARD for i in range(N_CORES - 1)] + [N_TOTAL - SHARD]
    in_maps = [dict(consts, x=_shard_x(X[s:s + SHARD])) for s in starts]

    nc = _get_nc()
    kw = {}
    if _trace:
        kw = dict(trace=True, **(_trace_kwargs or {}))
    res = run_bass_kernel_spmd(nc, in_maps, list(range(N_CORES)), **kw)

    out = np.empty(N_TOTAL, dtype=np.float32)
    for i, s in enumerate(starts):
        out[s:s + SHARD] = res.results[i]["out"].reshape(-1)
    if _trace:
        return out, res
    return out


# revision 35
# speedup vs baseline: 6.2790x; 6.2790x over previous
"""Trainium2 Bass kernel for CustomRBF forward:

    out[i] = w * exp(-gamma * ||X[i] - centroid||^2) + b

Flat-contiguous layout (per core, data-parallel over 8 cores):
  - The 125056-sample shard is viewed flat as [128 partitions, 977 samples
    * 128 feats]: partition p holds samples [p*977, (p+1)*977) of the
    shard, each partition line a fully CONTIGUOUS 500,224-byte DRAM run.
    DMA groups of `ch` sample-columns load [128, ch*128] tiles whose
    per-partition reads are ch*512 contiguous bytes (32 KB at ch=64) --
    vs the 512-byte strided chunks of the natural [sample, feat] layout.
  - PE path per 128-sample column slice: TensorE transpose [p, 128] ->
    PSUM [feat, p]; ScalarE fused subtract+square (activation Square,
    bias=-c, per-partition = per-feature); TensorE fp32r matmul (squared
    slice stationary, ones moving) reduces over feature partitions ->
    one PSUM accumulator column [128, 1] that lands DIRECTLY in the
    [partition, sample-col] output layout (no transpose-back needed).
  - Optional DVE path (ratio knob) keeps whole slices on VectorE
    (tensor_sub/mul/reduce in natural flat layout) to rebalance engines.
  - Finalize once per repeat per path: ScalarE Exp (scale=-gamma) PSUM ->
    SBUF, VectorE tensor_scalar (*w + b), one output DMA per run of
    consecutive columns ([128, run] -> out[p*977 + c]).

Sharding: cores 0-6 take contiguous 125056-sample slices; core 7 takes the
last 125056 samples (overlapping core 6 by 448 samples; the overlap is
recomputed identically and overwritten at gather time).

`repeats` re-emits the whole pipeline R times in one NEFF (same data, same
output) -- used only for differential wall-clock timing of the steady state.
"""

import sys

sys.path.insert(0, "/opt/trn_rl_repo")

import numpy as np

D = 128          # feature dim
P = 128          # SBUF partitions
GAMMA = 1.0 / D
N_CORES = 8
SPP = 977        # samples per partition
SHARD = P * SPP  # 125056
N_TOTAL = 1000000
CH = 64          # sample-columns per input DMA
HG = 8           # sample-columns per half-group (one ACT square)

_NC_CACHE = {}

# kernel() build config (current best known)
BEST = dict(xin_bufs=4, depth=3, pe_num=9, den=16, vscalar=1)


def _build(spp=SPP, repeats=1, ch=CH, hg=HG, xin_bufs=3, y_bufs=3,
           tr_bufs=2, acc_bufs=1, nmov=2, pe_num=1, den=1, depth=2,
           stage="full", dma_eng="sync", ydt="f32r", out_eng="gpsimd",
           xdt="f32", vscalar=0):
    from contextlib import ExitStack

    import concourse.tile as tile
    from concourse import bacc, mybir

    f32 = mybir.dt.float32
    f32r = mybir.dt.float32r
    ydtype = {"f32r": f32r, "bf16": mybir.dt.bfloat16}[ydt]
    xdtype = {"f32": f32, "f32r": f32r}[xdt]
    # y tiles must outlive the matmul deferral window
    y_bufs = max(y_bufs, depth + 2)
    Act = mybir.ActivationFunctionType
    Alu = mybir.AluOpType

    nc = bacc.Bacc("TRN2", target_bir_lowering=False, debug=False,
                   num_devices=N_CORES)
    xh = nc.declare_dram_parameter("x", [P, spp * D], xdtype, isOutput=False)
    negch = nc.declare_dram_parameter("negc", [P, 1], f32, isOutput=False)
    identh = nc.declare_dram_parameter("ident", [P, D], f32, isOutput=False)
    onesh = nc.declare_dram_parameter("ones", [P, 2], f32, isOutput=False)
    creph = nc.declare_dram_parameter("crep", [P, HG * D], f32,
                                      isOutput=False)
    wh = nc.declare_dram_parameter("wvec", [P, 1], f32, isOutput=False)
    bh = nc.declare_dram_parameter("bvec", [P, 1], f32, isOutput=False)
    outh = nc.declare_dram_parameter("out", [P, spp], f32, isOutput=True)

    with ExitStack() as ctx:
        tc = ctx.enter_context(tile.TileContext(nc))
        singles = ctx.enter_context(tc.tile_pool(name="singles", bufs=1))
        xin = ctx.enter_context(tc.tile_pool(name="xin", bufs=xin_bufs))
        yp = ctx.enter_context(tc.tile_pool(name="y", bufs=y_bufs))
        dfp = ctx.enter_context(tc.tile_pool(name="df", bufs=2))
        resp = ctx.enter_context(tc.tile_pool(name="res", bufs=2))
        trp = ctx.enter_context(tc.tile_pool(name="tr", bufs=tr_bufs,
                                             space="PSUM"))
        accp = ctx.enter_context(tc.tile_pool(name="acc", bufs=acc_bufs,
                                              space="PSUM"))

        negc_s = singles.tile([P, 1], f32)
        nc.sync.dma_start(out=negc_s, in_=negch[:, :])
        ident_s = singles.tile([P, D], f32)
        nc.sync.dma_start(out=ident_s, in_=identh[:, :])
        if xdtype is not f32:
            ident_x = singles.tile([P, D], xdtype)
            nc.vector.tensor_copy(out=ident_x, in_=ident_s)
        else:
            ident_x = ident_s
        ones_s = singles.tile([P, 2], f32)
        nc.sync.dma_start(out=ones_s, in_=onesh[:, :])
        ones_r = singles.tile([P, 2], ydtype)
        nc.vector.tensor_copy(out=ones_r, in_=ones_s)
        crep_s = singles.tile([P, HG * D], f32)
        nc.sync.dma_start(out=crep_s, in_=creph[:, :])
        crep3 = crep_s.rearrange("p (t k) -> p t k", k=D)
        wv_s = singles.tile([P, 1], f32)
        nc.sync.dma_start(out=wv_s, in_=wh[:, :])
        bv_s = singles.tile([P, 1], f32)
        nc.sync.dma_start(out=bv_s, in_=bh[:, :])

        # per-repeat state
        state = {}

        def start_repeat():
            state["acc"] = accp.tile([P, spp * nmov], f32, name="acc",
                                     tag="acc")
            state["pending"] = []   # deferred matmul closures

        def acc_view(acc, c0, n):
            if nmov == 2:
                return acc.rearrange("p (t two) -> p t two",
                                     two=2)[:, c0:c0 + n, 0:1]
            return acc[:, c0:c0 + n]

        def finalize():
            acc = state["acc"]
            res = resp.tile([P, spp], f32, name="res", tag="res")
            nc.scalar.activation(out=res[:, :], in_=acc_view(acc, 0, spp),
                                 func=Act.Exp, scale=-GAMMA, bias=0.0)
            nc.vector.tensor_scalar(out=res[:, :], in0=res[:, :],
                                    scalar1=wv_s[:, :], scalar2=bv_s[:, :],
                                    op0=Alu.mult, op1=Alu.add)
            oeng = {"sync": nc.sync, "scalar": nc.scalar,
                    "gpsimd": nc.gpsimd}[out_eng]
            oeng.dma_start(out=outh[:, :], in_=res[:, :])

        def flush(all_=False):
            q = state["pending"]
            while q and (all_ or len(q) > depth):
                q.pop(0)()

        def pe_half(xt3, h, hw, col):
            tr = trp.tile([P, hg * D], xdtype, name="tr", tag="tr")
            for j in range(hw):
                nc.tensor.transpose(out=tr[:, j * D:(j + 1) * D],
                                    in_=xt3[:, h + j, :],
                                    identity=ident_x[:, :])
            y = yp.tile([P, hg * D], ydtype, name="y", tag="y")
            nc.scalar.activation(out=y[:, :hw * D], in_=tr[:, :hw * D],
                                 func=Act.Square, bias=negc_s[:, :],
                                 scale=1.0)
            if stage == "sq":
                return
            acc = state["acc"]

            def back():
                for j in range(hw):
                    c = (col + j) * nmov
                    nc.tensor.matmul(out=acc[:, c:c + nmov],
                                     lhsT=y[:, j * D:(j + 1) * D],
                                     rhs=ones_r[:, :nmov],
                                     start=True, stop=True)

            state["pending"].append(back)

        def v_half(xt3, h, hw, col):
            df = dfp.tile([P, hg, D], f32, name="df", tag="df")
            nc.vector.tensor_sub(out=df[:, :hw, :], in0=xt3[:, h:h + hw, :],
                                 in1=crep3[:, :hw, :])
            if vscalar:
                nc.scalar.activation(out=df[:, :hw, :], in_=df[:, :hw, :],
                                     func=Act.Square, bias=0.0, scale=1.0)
            else:
                nc.vector.tensor_mul(out=df[:, :hw, :], in0=df[:, :hw, :],
                                     in1=df[:, :hw, :])
            nc.vector.tensor_reduce(out=acc_view(state["acc"], col, hw),
                                    in_=df[:, :hw, :],
                                    axis=mybir.AxisListType.X, op=Alu.add)

        grp_idx = 0
        for _rep in range(repeats):
            start_repeat()
            hg_idx = 0
            col = 0
            while col < spp:
                cw = min(ch, spp - col)
                xt = xin.tile([P, ch * D], xdtype, name="xt", tag="xt")
                if stage == "nodma":
                    # tiny writer so the tile has a producer; compute reads
                    # mostly-stale SBUF (timing probe only)
                    nc.sync.dma_start(out=xt[:, 0:D], in_=xh[:, 0:D])
                elif dma_eng == "split":
                    half = (cw // 2) * D
                    nc.sync.dma_start(out=xt[:, :half],
                                      in_=xh[:, col * D:col * D + half])
                    nc.scalar.dma_start(
                        out=xt[:, half:cw * D],
                        in_=xh[:, col * D + half:(col + cw) * D])
                elif dma_eng == "split3":
                    c3 = [0, (cw // 3) * D, (2 * (cw // 3)) * D, cw * D]
                    for eng, a, bnd in ((nc.sync, c3[0], c3[1]),
                                        (nc.scalar, c3[1], c3[2]),
                                        (nc.gpsimd, c3[2], c3[3])):
                        eng.dma_start(out=xt[:, a:bnd],
                                      in_=xh[:, col * D + a:col * D + bnd])
                elif dma_eng == "split_sg":
                    half = (cw // 2) * D
                    nc.sync.dma_start(out=xt[:, :half],
                                      in_=xh[:, col * D:col * D + half])
                    nc.gpsimd.dma_start(
                        out=xt[:, half:cw * D],
                        in_=xh[:, col * D + half:(col + cw) * D])
                else:
                    eng = {"sync": nc.sync, "gpsimd": nc.gpsimd}.get(
                        dma_eng)
                    if eng is None:  # alt
                        eng = nc.sync if grp_idx % 2 == 0 else nc.scalar
                    eng.dma_start(out=xt[:, :cw * D],
                                  in_=xh[:, col * D:(col + cw) * D])
                grp_idx += 1
                xt3 = xt.rearrange("p (s k) -> p s k", k=D)
                h = 0
                while h < cw and stage != "dma":
                    hw = min(hg, cw - h)
                    is_pe = (((hg_idx + 1) * pe_num) // den
                             > (hg_idx * pe_num) // den)
                    if is_pe:
                        flush()
                        pe_half(xt3, h, hw, col + h)
                    else:
                        v_half(xt3, h, hw, col + h)
                    hg_idx += 1
                    h += hw
                col += cw
            flush(all_=True)
            if stage == "full":
                finalize()
        if stage != "full":
            # keep the output tensor written so the NEFF has a producer
            nc.sync.dma_start(out=outh[:, 0:D], in_=ident_s[:, :])

    nc.finalize()
    return nc


def _get_nc():
    if "nc" not in _NC_CACHE:
        _NC_CACHE["nc"] = _build(**BEST)
    return _NC_CACHE["nc"]


def _make_const_inputs(centroid, w, b):
    centroid = np.asarray(centroid, dtype=np.float32).reshape(D)
    w = np.asarray(w, dtype=np.float32).reshape(-1)[0]
    b = np.asarray(b, dtype=np.float32).reshape(-1)[0]
    return {
        "negc": (-centroid).reshape(P, 1).copy(),
        "ident": np.eye(P, dtype=np.float32),
        "ones": np.tile(np.array([1.0, 0.0], dtype=np.float32), (P, 1)),
        "crep": np.tile(np.tile(centroid, HG), (P, 1)),
        "wvec": np.full((P, 1), w, dtype=np.float32),
        "bvec": np.full((P, 1), b, dtype=np.float32),
    }


def _shard_x(x_shard):
    # [SHARD, D] sample-major -> flat [P, SPP*D]: partition p holds
    # samples [p*SPP, (p+1)*SPP) as one contiguous run (pure view).
    return np.ascontiguousarray(x_shard).reshape(P, SPP * D)


def kernel(X, centroid, w, b, _trace=False, _trace_kwargs=None):
    from concourse.bass_utils import run_bass_kernel_spmd

    X = np.asarray(X)
    assert X.shape == (N_TOTAL, D), X.shape
    if X.dtype != np.float32:
        X = X.astype(np.float32)

    consts = _make_const_inputs(centroid, w, b)
    starts = [i * SHARD for i in range(N_CORES - 1)] + [N_TOTAL - SHARD]
    in_maps = [dict(consts, x=_shard_x(X[s:s + SHARD])) for s in starts]

    nc = _get_nc()
    kw = {}
    if _trace:
        kw = dict(trace=True, **(_trace_kwargs or {}))
    res = run_bass_kernel_spmd(nc, in_maps, list(range(N_CORES)), **kw)

    out = np.empty(N_TOTAL, dtype=np.float32)
    for i, s in enumerate(starts):
        out[s:s + SHARD] = res.results[i]["out"].reshape(-1)
    if _trace:
        return out, res
    return out


# revision 39
# speedup vs baseline: 12.3891x; 1.9731x over previous
"""Trainium2 Bass kernel for CustomRBF forward:

    out[i] = w * exp(-gamma * ||X[i] - centroid||^2) + b

Flat-contiguous layout (per core, data-parallel over 8 cores):
  - The 125056-sample shard is viewed flat as [128 partitions, 977 samples
    * 128 feats]: partition p holds samples [p*977, (p+1)*977) of the
    shard, each partition line a fully CONTIGUOUS 500,224-byte DRAM run.
    DMA groups of `ch` sample-columns load [128, ch*128] tiles whose
    per-partition reads are ch*512 contiguous bytes (32 KB at ch=64) --
    vs the 512-byte strided chunks of the natural [sample, feat] layout.
  - PE path per 128-sample column slice: TensorE transpose [p, 128] ->
    PSUM [feat, p]; ScalarE fused subtract+square (activation Square,
    bias=-c, per-partition = per-feature); TensorE fp32r matmul (squared
    slice stationary, ones moving) reduces over feature partitions ->
    one PSUM accumulator column [128, 1] that lands DIRECTLY in the
    [partition, sample-col] output layout (no transpose-back needed).
  - DVE path (7 of every 16 half-groups, pe_num/den knob): VectorE
    tensor_sub in natural flat layout, ScalarE Square (vscalar=1), VectorE
    segmented tensor_reduce writing the SAME strided PSUM accumulator --
    rebalances the TensorE-bound PE path (~300ns/slice) against otherwise
    idle VectorE (~280ns/slice for sub+reduce).
  - Finalize once per repeat: ScalarE Exp (scale=-gamma) over the strided
    accumulator PSUM -> SBUF, VectorE tensor_scalar (*w + b), one 500KB
    output DMA [128, 977] (issued on gpsimd/SWDGE to keep the sync HWDGE
    ring free for input DMAs).

Sharding: cores 0-6 take contiguous 125056-sample slices; core 7 takes the
last 125056 samples (overlapping core 6 by 448 samples; the overlap is
recomputed identically and overwritten at gather time).

`repeats` re-emits the whole pipeline R times in one NEFF (same data, same
output) -- used only for differential wall-clock timing of the steady state.
"""

import sys

sys.path.insert(0, "/opt/trn_rl_repo")

import numpy as np

D = 128          # feature dim
P = 128          # SBUF partitions
GAMMA = 1.0 / D
N_CORES = 8
SPP = 977        # samples per partition
SHARD = P * SPP  # 125056
N_TOTAL = 1000000
CH = 64          # sample-columns per input DMA
HG = 8           # sample-columns per half-group (one ACT square)

_NC_CACHE = {}

# kernel() build config (current best known)
BEST = dict(ch=112, xin_bufs=2, depth=3, pe_num=9, den=16, vscalar=1,
            df_bufs=4)


def _build(spp=SPP, repeats=1, ch=CH, hg=HG, xin_bufs=3, y_bufs=3,
           tr_bufs=2, acc_bufs=1, nmov=2, pe_num=1, den=1, depth=2,
           stage="full", dma_eng="sync", ydt="f32r", out_eng="gpsimd",
           xdt="f32", vscalar=0, df_bufs=2):
    from contextlib import ExitStack

    import concourse.tile as tile
    from concourse import bacc, mybir

    f32 = mybir.dt.float32
    f32r = mybir.dt.float32r
    ydtype = {"f32r": f32r, "bf16": mybir.dt.bfloat16}[ydt]
    xdtype = {"f32": f32, "f32r": f32r}[xdt]
    # y tiles must outlive the matmul deferral window
    y_bufs = max(y_bufs, depth + 2)
    Act = mybir.ActivationFunctionType
    Alu = mybir.AluOpType

    nc = bacc.Bacc("TRN2", target_bir_lowering=False, debug=False,
                   num_devices=N_CORES)
    xh = nc.declare_dram_parameter("x", [P, spp * D], xdtype, isOutput=False)
    negch = nc.declare_dram_parameter("negc", [P, 1], f32, isOutput=False)
    identh = nc.declare_dram_parameter("ident", [P, D], f32, isOutput=False)
    onesh = nc.declare_dram_parameter("ones", [P, 2], f32, isOutput=False)
    creph = nc.declare_dram_parameter("crep", [P, HG * D], f32,
                                      isOutput=False)
    wh = nc.declare_dram_parameter("wvec", [P, 1], f32, isOutput=False)
    bh = nc.declare_dram_parameter("bvec", [P, 1], f32, isOutput=False)
    outh = nc.declare_dram_parameter("out", [P, spp], f32, isOutput=True)

    with ExitStack() as ctx:
        tc = ctx.enter_context(tile.TileContext(nc))
        singles = ctx.enter_context(tc.tile_pool(name="singles", bufs=1))
        xin = ctx.enter_context(tc.tile_pool(name="xin", bufs=xin_bufs))
        yp = ctx.enter_context(tc.tile_pool(name="y", bufs=y_bufs))
        dfp = ctx.enter_context(tc.tile_pool(name="df", bufs=df_bufs))
        resp = ctx.enter_context(tc.tile_pool(name="res", bufs=2))
        trp = ctx.enter_context(tc.tile_pool(name="tr", bufs=tr_bufs,
                                             space="PSUM"))
        accp = ctx.enter_context(tc.tile_pool(name="acc", bufs=acc_bufs,
                                              space="PSUM"))

        negc_s = singles.tile([P, 1], f32)
        nc.sync.dma_start(out=negc_s, in_=negch[:, :])
        ident_s = singles.tile([P, D], f32)
        nc.sync.dma_start(out=ident_s, in_=identh[:, :])
        if xdtype is not f32:
            ident_x = singles.tile([P, D], xdtype)
            nc.vector.tensor_copy(out=ident_x, in_=ident_s)
        else:
            ident_x = ident_s
        ones_s = singles.tile([P, 2], f32)
        nc.sync.dma_start(out=ones_s, in_=onesh[:, :])
        ones_r = singles.tile([P, 2], ydtype)
        nc.vector.tensor_copy(out=ones_r, in_=ones_s)
        crep_s = singles.tile([P, HG * D], f32)
        nc.sync.dma_start(out=crep_s, in_=creph[:, :])
        crep3 = crep_s.rearrange("p (t k) -> p t k", k=D)
        wv_s = singles.tile([P, 1], f32)
        nc.sync.dma_start(out=wv_s, in_=wh[:, :])
        bv_s = singles.tile([P, 1], f32)
        nc.sync.dma_start(out=bv_s, in_=bh[:, :])

        # per-repeat state
        state = {}

        def start_repeat():
            state["acc"] = accp.tile([P, spp * nmov], f32, name="acc",
                                     tag="acc")
            state["pending"] = []   # deferred matmul closures

        def acc_view(acc, c0, n):
            if nmov == 2:
                return acc.rearrange("p (t two) -> p t two",
                                     two=2)[:, c0:c0 + n, 0:1]
            return acc[:, c0:c0 + n]

        def finalize():
            acc = state["acc"]
            res = resp.tile([P, spp], f32, name="res", tag="res")
            nc.scalar.activation(out=res[:, :], in_=acc_view(acc, 0, spp),
                                 func=Act.Exp, scale=-GAMMA, bias=0.0)
            nc.vector.tensor_scalar(out=res[:, :], in0=res[:, :],
                                    scalar1=wv_s[:, :], scalar2=bv_s[:, :],
                                    op0=Alu.mult, op1=Alu.add)
            oeng = {"sync": nc.sync, "scalar": nc.scalar,
                    "gpsimd": nc.gpsimd}[out_eng]
            oeng.dma_start(out=outh[:, :], in_=res[:, :])

        def flush(all_=False):
            q = state["pending"]
            while q and (all_ or len(q) > depth):
                q.pop(0)()

        def pe_half(xt3, h, hw, col):
            tr = trp.tile([P, hg * D], xdtype, name="tr", tag="tr")
            for j in range(hw):
                nc.tensor.transpose(out=tr[:, j * D:(j + 1) * D],
                                    in_=xt3[:, h + j, :],
                                    identity=ident_x[:, :])
            y = yp.tile([P, hg * D], ydtype, name="y", tag="y")
            nc.scalar.activation(out=y[:, :hw * D], in_=tr[:, :hw * D],
                                 func=Act.Square, bias=negc_s[:, :],
                                 scale=1.0)
            if stage == "sq":
                return
            acc = state["acc"]

            def back():
                for j in range(hw):
                    c = (col + j) * nmov
                    nc.tensor.matmul(out=acc[:, c:c + nmov],
                                     lhsT=y[:, j * D:(j + 1) * D],
                                     rhs=ones_r[:, :nmov],
                                     start=True, stop=True)

            state["pending"].append(back)

        def v_half(xt3, h, hw, col):
            df = dfp.tile([P, hg, D], f32, name="df", tag="df")
            nc.vector.tensor_sub(out=df[:, :hw, :], in0=xt3[:, h:h + hw, :],
                                 in1=crep3[:, :hw, :])
            if vscalar:
                nc.scalar.activation(out=df[:, :hw, :], in_=df[:, :hw, :],
                                     func=Act.Square, bias=0.0, scale=1.0)
            else:
                nc.vector.tensor_mul(out=df[:, :hw, :], in0=df[:, :hw, :],
                                     in1=df[:, :hw, :])
            nc.vector.tensor_reduce(out=acc_view(state["acc"], col, hw),
                                    in_=df[:, :hw, :],
                                    axis=mybir.AxisListType.X, op=Alu.add)

        grp_idx = 0
        for _rep in range(repeats):
            start_repeat()
            hg_idx = 0
            col = 0
            while col < spp:
                cw = min(ch, spp - col)
                xt = xin.tile([P, ch * D], xdtype, name="xt", tag="xt")
                if stage == "nodma":
                    # tiny writer so the tile has a producer; compute reads
                    # mostly-stale SBUF (timing probe only)
                    nc.sync.dma_start(out=xt[:, 0:D], in_=xh[:, 0:D])
                elif dma_eng == "split":
                    half = (cw // 2) * D
                    nc.sync.dma_start(out=xt[:, :half],
                                      in_=xh[:, col * D:col * D + half])
                    nc.scalar.dma_start(
                        out=xt[:, half:cw * D],
                        in_=xh[:, col * D + half:(col + cw) * D])
                elif dma_eng == "split3":
                    c3 = [0, (cw // 3) * D, (2 * (cw // 3)) * D, cw * D]
                    for eng, a, bnd in ((nc.sync, c3[0], c3[1]),
                                        (nc.scalar, c3[1], c3[2]),
                                        (nc.gpsimd, c3[2], c3[3])):
                        eng.dma_start(out=xt[:, a:bnd],
                                      in_=xh[:, col * D + a:col * D + bnd])
                elif dma_eng == "split_sg":
                    half = (cw // 2) * D
                    nc.sync.dma_start(out=xt[:, :half],
                                      in_=xh[:, col * D:col * D + half])
                    nc.gpsimd.dma_start(
                        out=xt[:, half:cw * D],
                        in_=xh[:, col * D + half:(col + cw) * D])
                else:
                    eng = {"sync": nc.sync, "gpsimd": nc.gpsimd}.get(
                        dma_eng)
                    if eng is None:  # alt
                        eng = nc.sync if grp_idx % 2 == 0 else nc.scalar
                    eng.dma_start(out=xt[:, :cw * D],
                                  in_=xh[:, col * D:(col + cw) * D])
                grp_idx += 1
                xt3 = xt.rearrange("p (s k) -> p s k", k=D)
                h = 0
                while h < cw and stage != "dma":
                    hw = min(hg, cw - h)
                    is_pe = (((hg_idx + 1) * pe_num) // den
                             > (hg_idx * pe_num) // den)
                    if is_pe:
                        flush()
                        pe_half(xt3, h, hw, col + h)
                    else:
                        v_half(xt3, h, hw, col + h)
                    hg_idx += 1
                    h += hw
                col += cw
            flush(all_=True)
            if stage == "full":
                finalize()
        if stage != "full":
            # keep the output tensor written so the NEFF has a producer
            nc.sync.dma_start(out=outh[:, 0:D], in_=ident_s[:, :])

    nc.finalize()
    return nc


def _get_nc():
    if "nc" not in _NC_CACHE:
        _NC_CACHE["nc"] = _build(**BEST)
    return _NC_CACHE["nc"]


def _make_const_inputs(centroid, w, b):
    centroid = np.asarray(centroid, dtype=np.float32).reshape(D)
    w = np.asarray(w, dtype=np.float32).reshape(-1)[0]
    b = np.asarray(b, dtype=np.float32).reshape(-1)[0]
    return {
        "negc": (-centroid).reshape(P, 1).copy(),
        "ident": np.eye(P, dtype=np.float32),
        "ones": np.tile(np.array([1.0, 0.0], dtype=np.float32), (P, 1)),
        "crep": np.tile(np.tile(centroid, HG), (P, 1)),
        "wvec": np.full((P, 1), w, dtype=np.float32),
        "bvec": np.full((P, 1), b, dtype=np.float32),
    }


def _shard_x(x_shard):
    # [SHARD, D] sample-major -> flat [P, SPP*D]: partition p holds
    # samples [p*SPP, (p+1)*SPP) as one contiguous run (pure view).
    return np.ascontiguousarray(x_shard).reshape(P, SPP * D)


def kernel(X, centroid, w, b, _trace=False, _trace_kwargs=None):
    from concourse.bass_utils import run_bass_kernel_spmd

    X = np.asarray(X)
    assert X.shape == (N_TOTAL, D), X.shape
    if X.dtype != np.float32:
        X = X.astype(np.float32)

    consts = _make_const_inputs(centroid, w, b)
    starts = [i * SHARD for i in range(N_CORES - 1)] + [N_TOTAL - SHARD]
    in_maps = [dict(consts, x=_shard_x(X[s:s + SHARD])) for s in starts]

    nc = _get_nc()
    kw = {}
    if _trace:
        kw = dict(trace=True, **(_trace_kwargs or {}))
    res = run_bass_kernel_spmd(nc, in_maps, list(range(N_CORES)), **kw)

    out = np.empty(N_TOTAL, dtype=np.float32)
    for i, s in enumerate(starts):
        out[s:s + SHARD] = res.results[i]["out"].reshape(-1)
    if _trace:
        return out, res
    return out
